# revision 15
# baseline (speedup 1.0000x reference)
"""Causal self-attention (B=2, T=2048, C=1024, H=16) on 8 trn2 NeuronCores.

Sharding: core c handles batch b=c//4 and head group g=c%4 (4 heads each).
Data parallel on B, tensor parallel on H; W_attn/W_proj sliced per head
group; host sums the 4 tensor-parallel partial projection outputs per batch.

v2 schedule (vs the phase-serial v1): the kernel is bound by the PE's
streamed-column count (~274k cycles) with the ACT exp stream (~79us)
second; everything else must hide under those.  The p-state ramp makes any
PE stall expensive (half clock for ~3us), so emission order keeps the PE
queue saturated:
  - W_qk columns are reordered host-side so j-tiles come as
    [q(h0,h1) | k(h0,h1) | q(h2,h3) | k(h2,h3)]: attention for heads 0,1
    of pair 0 starts right after the first half of the qkT GEMM.
  - attention steps are software-pipelined S(g) -> exp(g) -> PV(g) with the
    remaining qkT j-tiles, the v GEMM groups, and pair-0 projection tiles
    injected between steps as PE filler while ACT chews exp.
  - one PSUM tag layout for all phases: 'big' [128,1024]x3 (qkT groups,
    v groups, S tiles, proj tiles) + 'y' [128,512]x2 (PV accumulators).
  - ACT runs ONLY Exp (one table load ever); all psum->sbuf copies and the
    qk bias add run on DVE (tensor_scalar_add with per-partition bias AP).
  - causal diag masking accumulates -448 into the mixed 128x128 block via an
    fp8e4 DoubleRow matmul (half the PE cycles of the bf16 version); exp's
    1/8 scale turns that into a -56 logit shift -> exact zeros.
  - S tail matmuls are exact-width (no widening), output is written bf16
    (halves the store DMA); host accumulates the 4 TP partials in f32.
Numerics match v1: bf16 operands, f32 psum, softmax denominator from a
ones column in v (psum row 0), reciprocal_approx_fast, no row-max.
"""

import os
import numpy as np
import ml_dtypes

import concourse.bacc as bacc
import concourse.mybir as mybir
import concourse.tile as tile
from concourse.bass_utils import run_bass_kernel_spmd
from concourse.masks import make_identity, make_lower_triangular

FP8_MASK = os.environ.get("KFP8MASK", "0") == "1"

B, T, C, H = 2, 2048, 1024, 16
D = C // H          # 64
HPC = H // 4        # 4 heads per core
QK = 2 * HPC * D    # 512 rows of qkT
V = HPC * D         # 256 v columns
F32 = mybir.dt.float32
BF16 = mybir.dt.bfloat16
FP8 = mybir.dt.float8e4
PAIR = 1024         # queries per attention pass
AF = mybir.ActivationFunctionType
DR = mybir.MatmulPerfMode.DoubleRow

_cache = {}


def _build():
    nc = bacc.Bacc("TRN2", target_bir_lowering=False, debug=False, num_devices=8)
    xT = nc.dram_tensor("xT", [C, T], BF16, kind="ExternalInput").ap()
    w_qk = nc.dram_tensor("w_qk", [C, QK], BF16, kind="ExternalInput").ap()
    b_qk = nc.dram_tensor("b_qk", [128, 4], F32, kind="ExternalInput").ap()
    w_v = nc.dram_tensor("w_v", [C, V], BF16, kind="ExternalInput").ap()
    b_v = nc.dram_tensor("b_v", [1, V], F32, kind="ExternalInput").ap()
    w_pr = nc.dram_tensor("w_pr", [V, C], BF16, kind="ExternalInput").ap()
    m8 = nc.dram_tensor("m8", [64, 512], FP8, kind="ExternalInput").ap()
    out = nc.dram_tensor("out", [T, C], BF16, kind="ExternalOutput").ap()

    NC_ = C // 128  # 8 c-tiles

    with tile.TileContext(nc) as tc:
        with (
            tc.tile_pool(name="const", bufs=1) as cpool,
            tc.tile_pool(name="xt", bufs=1) as xpool,
            tc.tile_pool(name="w", bufs=1) as wpool,
            tc.tile_pool(name="qk", bufs=1) as qkpool,
            tc.tile_pool(name="v", bufs=1) as vpool,
            tc.tile_pool(name="att", bufs=1) as apool,
            tc.tile_pool(name="o", bufs=1) as opool,
            tc.tile_pool(name="ps", bufs=1, space="PSUM") as ps,
        ):
            # ---- input DMA (each trigger costs ~600ns of sync-queue time,
            # so whole weights go as single strided transfers) ----
            wqk_sb = wpool.tile([128, NC_, QK], BF16, name="wqk")
            # j-tiles 0,1 first: the opening qkT groups only need them
            nc.sync.dma_start(wqk_sb[:, :, 0:256],
                              w_qk[:, 0:256].rearrange("(a p) j -> p a j", p=128))
            xt = []
            for c in range(NC_):
                t = xpool.tile([128, T], BF16, name=f"xt{c}")
                nc.sync.dma_start(t[:], xT[c * 128:(c + 1) * 128, :])
                xt.append(t)
                if c == 2:
                    bqk_sb = cpool.tile([128, 4], F32, name="bqk")
                    nc.sync.dma_start(bqk_sb[:], b_qk[:, :])
                    m8_sb = cpool.tile([64, 512], FP8, name="m8")
                    nc.sync.dma_start(m8_sb[:], m8[:, :])
                    bv_row = cpool.tile([1, V], F32, name="bv_row")
                    nc.sync.dma_start(bv_row[:], b_v[:])
            nc.sync.dma_start(wqk_sb[:, :, 256:512],
                              w_qk[:, 256:512].rearrange("(a p) j -> p a j", p=128))
            wv_sb = wpool.tile([128, NC_, V], BF16, name="wv")
            nc.sync.dma_start(wv_sb[:], w_v.rearrange("(a p) v -> p a v", p=128))
            wpr_sb = wpool.tile([128, 2, C], BF16, name="wpr")
            nc.sync.dma_start(wpr_sb[:], w_pr.rearrange("(k p) c -> p k c", p=128))

            if FP8_MASK:
                ident_pack = m8_sb[:, 0:256].rearrange("p (a c) -> p a c", a=2)
                trineg_pack = m8_sb[:, 256:512].rearrange("p (a c) -> p a c", a=2)
            else:
                ident_bf = cpool.tile([128, 128], BF16, name="ident_bf")
                make_identity(nc, ident_bf[:])
                trineg_bf = cpool.tile([128, 128], BF16, name="trineg_bf")
                make_lower_triangular(nc, trineg_bf[:], val=-8192.0, diag=False)
            bv_full = cpool.tile([128, V], F32, name="bv_full")
            nc.gpsimd.partition_broadcast(bv_full[:], bv_row[:])
            ones16 = cpool.tile([128, 16], F32, name="ones16")
            nc.gpsimd.memset(ones16[:], 1.0)

            qk_t = [qkpool.tile([128, T], BF16, name=f"qk{j}")
                    for j in range(QK // 128)]
            # v layout: 4 big tiles of 4 T-tiles each; per (tt,h) 128 cols:
            # col 0 = ones (softmax denominator -> psum row 0), cols 64..127
            # = v dims (y -> psum rows 64..127, 64-aligned).
            v_big = [vpool.tile([128, 4, HPC, 128], BF16, name=f"v{b_}")
                     for b_ in range(4)]
            for b_ in range(4):
                # cols 1..63 are never read downstream; zero them once so
                # the PV lhsT reads fully-initialized memory (CoreSim).
                nc.gpsimd.memset(v_big[b_][:, :, :, 1:64], 0.0)

            def qk_group(j, tp):
                pst = ps.tile([128, 1024], F32, name="qk_ps", tag="big", bufs=3)
                for c in range(NC_):
                    for s in range(2):
                        nc.tensor.matmul(
                            pst[:, s * 512:(s + 1) * 512],
                            wqk_sb[:, c, j * 128:(j + 1) * 128],
                            xt[c][:, tp * 1024 + s * 512:tp * 1024 + (s + 1) * 512],
                            start=(c == 0), stop=(c == NC_ - 1))
                nc.vector.tensor_scalar_add(
                    qk_t[j][:, tp * 1024:(tp + 1) * 1024], pst[:],
                    bqk_sb[:, j:j + 1])

            def v_group(vb):
                pst = ps.tile([128, 1024], F32, name="v_ps", tag="big", bufs=3)
                for tl in range(4):
                    tt = vb * 4 + tl
                    for c in range(NC_):
                        nc.tensor.matmul(
                            pst[:, tl * 256:(tl + 1) * 256],
                            xt[c][:, tt * 128:(tt + 1) * 128],
                            wv_sb[:, c, :],
                            start=(c == 0), stop=(c == NC_ - 1))
                for tl in range(4):
                    nc.vector.tensor_add(
                        v_big[vb][:, tl, :, 64:64 + D],
                        pst[:, tl * 256:(tl + 1) * 256]
                        .rearrange("p (h d) -> p h d", h=HPC),
                        bv_full[:].rearrange("p (h d) -> p h d", h=HPC))
                nc.vector.tensor_copy(
                    v_big[vb][:, :, :, 0:1],
                    ones16[:].rearrange("p (t h o) -> p t h o", t=4, h=HPC))

            def proj_group(p, tt):
                i0 = p * PAIR
                o_ps = ps.tile([128, 1024], F32, name="o_ps", tag="big", bufs=3)
                for cc in range(2):
                    for k in range(2):
                        nc.tensor.matmul(
                            o_ps[:, cc * 512:(cc + 1) * 512],
                            yn[p][k][:, tt * 128:(tt + 1) * 128],
                            wpr_sb[:, k, cc * 512:(cc + 1) * 512],
                            start=(k == 0), stop=(k == 1))
                osb = opool.tile([128, 1024], BF16, name="osb", tag="osb",
                                 bufs=3)
                nc.vector.tensor_copy(osb[:], o_ps[:])
                nc.sync.dma_start(
                    out[i0 + tt * 128:i0 + (tt + 1) * 128, :], osb[:])

            yn = {}

            def emit_s_exp(p, h, g):
                """S matmuls + exp for one (pair, head, j-tile); returns pT."""
                i0 = p * PAIR
                qrow = (h % 2) * D
                qtile = qk_t[2 * (h // 2)]
                ktile = qk_t[2 * (h // 2) + 1]
                j0 = g * 128
                dlt = max(0, j0 - i0)
                diag = j0 >= i0
                diag_sub = dlt // 512
                s_ps = ps.tile([128, PAIR], F32, name="s_ps", tag="big",
                               bufs=3)
                pT = apool.tile([128, PAIR], BF16, name="pT", tag="pT",
                                bufs=11)
                for sub in range(2):
                    lo = max(dlt, sub * 512)
                    if lo >= (sub + 1) * 512:
                        continue
                    nc.tensor.matmul(
                        s_ps[:, lo:(sub + 1) * 512],
                        ktile[qrow:qrow + D, j0:j0 + 128],
                        qtile[qrow:qrow + D, i0 + lo:i0 + (sub + 1) * 512],
                        start=True,
                        stop=not (diag and sub == diag_sub))
                if diag:
                    # accumulate a large negative value into the mixed
                    # diagonal block (ident.T @ trineg == trineg) so exp
                    # yields exact zeros above the diagonal.
                    if FP8_MASK:
                        nc.tensor.matmul(
                            s_ps[:, dlt:dlt + 128],
                            ident_pack, trineg_pack,
                            start=False, stop=True, perf_mode=DR)
                    else:
                        nc.tensor.matmul(
                            s_ps[:, dlt:dlt + 128],
                            ident_bf[:], trineg_bf[:],
                            start=False, stop=True)
                nc.scalar.activation(
                    pT[:, dlt:PAIR], s_ps[:, dlt:PAIR], AF.Exp,
                    scale=float(1.0 / np.sqrt(D)))
                return pT

            def emit_pv(p, h, g, pT, y_A, y_B):
                i0 = p * PAIR
                njt = (i0 + PAIR) // 128
                lastA = (i0 + 512) // 128 - 1
                dlt = max(0, g * 128 - i0)
                vsl = v_big[g // 4][:, g % 4, h, :]
                if dlt < 512:
                    nc.tensor.matmul(
                        y_A[:, dlt:512], vsl, pT[:, dlt:512],
                        start=(g == 0), stop=(g == lastA))
                loB = max(512, dlt)
                nc.tensor.matmul(
                    y_B[:, loB - 512:512], vsl, pT[:, loB:PAIR],
                    start=(g == 0), stop=(g == njt - 1))

            def alloc_y():
                return (ps.tile([128, 512], F32, name="y_A", tag="y", bufs=2),
                        ps.tile([128, 512], F32, name="y_B", tag="y", bufs=2))

            def emit_norm(p, h, y_A, y_B):
                # normalize: rows 64..127 divided by row 0 (l sums)
                qrow = (h % 2) * D
                rec = apool.tile([1, PAIR], F32, name="rec", tag="rec", bufs=2)
                nc.vector.reciprocal_approx_fast(rec[:, 0:512], y_A[0:1, :])
                nc.vector.reciprocal_approx_fast(rec[:, 512:PAIR], y_B[0:1, :])
                rb = apool.tile([D, PAIR], F32, name="rb", tag="rb", bufs=2)
                nc.gpsimd.partition_broadcast(rb[:, 0:512], rec[:, 0:512])
                nc.gpsimd.partition_broadcast(rb[:, 512:PAIR], rec[:, 512:PAIR])
                nc.vector.tensor_mul(
                    yn[p][h // 2][qrow:qrow + D, 0:512],
                    y_A[64:64 + D, :], rb[:, 0:512])
                nc.vector.tensor_mul(
                    yn[p][h // 2][qrow:qrow + D, 512:PAIR],
                    y_B[64:64 + D, :], rb[:, 512:PAIR])

            def attn_head_steps(p, h):
                """Yields once per j-tile step for (pair p, head h)."""
                njt = (p * PAIR + PAIR) // 128
                y_A, y_B = alloc_y()
                for g in range(njt):
                    pT = emit_s_exp(p, h, g)
                    emit_pv(p, h, g, pT, y_A, y_B)
                    yield
                emit_norm(p, h, y_A, y_B)

            def run_steps(heads, p, fillers, every):
                if 0 in heads:
                    yn[p] = [apool.tile([128, PAIR], BF16, name=f"yn{p}_{k}",
                                        tag=f"yn{k}", bufs=2)
                             for k in range(2)]
                n = 0
                for h in heads:
                    for _ in attn_head_steps(p, h):
                        n += 1
                        if fillers and n % every == 0:
                            fillers.pop(0)()
                    if fillers:
                        # head-boundary filler: the next head's first PV
                        # waits on this head's normalize chain (DVE) to free
                        # a y buffer; give the PE work in the meantime.
                        fillers.pop(0)()
                while fillers:
                    fillers.pop(0)()

            # ================= emission =================
            # minimum front before attention(p0,h0) can run: q/k of heads
            # 0,1 for queries 0..1023 (tp=0 halves) + v tiles 0..7.  The
            # exp stream on ACT is the kernel's second-binding resource, so
            # it must start as early as possible and never starve; all
            # remaining GEMM work is injected between attention steps as
            # PE filler while ACT chews exp.
            qk_group(0, 0)
            qk_group(1, 0)
            # h0-split: S+exp for (pair0, head0) depend only on the two qk
            # groups above, so they go ahead of the v GEMM — ACT starts
            # ~14us earlier and the v groups run under the exp stream.
            yn[0] = [apool.tile([128, PAIR], BF16, name=f"yn0_{k}",
                                tag=f"yn{k}", bufs=2) for k in range(2)]
            pts0 = [emit_s_exp(0, 0, g) for g in range(8)]
            v_group(0)
            v_group(1)
            y_A0, y_B0 = alloc_y()
            for g in range(8):
                emit_pv(0, 0, g, pts0[g], y_A0, y_B0)
            emit_norm(0, 0, y_A0, y_B0)
            fill0 = [lambda j=j: qk_group(j, 0) for j in (2, 3)]
            fill0 += [lambda vb=vb: v_group(vb) for vb in (2, 3)]
            run_steps([1], 0, fill0, every=3)
            fill0b = [lambda j=j: qk_group(j, 1) for j in (0, 1, 2, 3)]
            run_steps([2, 3], 0, fill0b, every=4)
            # pair 1, all heads; fillers: pair-0 projection
            fill1 = [lambda tt=tt: proj_group(0, tt) for tt in range(8)]
            run_steps([0, 1, 2, 3], 1, fill1, every=8)
            for tt in range(8):
                proj_group(1, tt)
    nc.compile()
    return nc


def _get_nc():
    if "nc" not in _cache:
        _cache["nc"] = _build()
    return _cache["nc"]


def kernel(x, W_attn, b_attn, W_proj, b_proj):
    x = np.asarray(x, dtype=np.float32)
    W_attn = np.asarray(W_attn, dtype=np.float32)
    b_attn = np.asarray(b_attn, dtype=np.float32)
    W_proj = np.asarray(W_proj, dtype=np.float32)
    b_proj = np.asarray(b_proj, dtype=np.float32)

    nc = _get_nc()

    # fp8 packed causal-mask constants: ident_pack | trineg_pack, [64, 2,128]
    r = np.arange(64)[:, None, None]
    a = np.arange(2)[None, :, None]
    i = np.arange(128)[None, None, :]
    ident = (i == 64 * a + r).astype(np.float32)
    # -240 is the max-magnitude finite value in BOTH e4m3 variants; with
    # exp's 1/8 scale it is a -30 logit shift -> P ~ e-13, effectively zero.
    trineg = np.where(64 * a + r > i, -240.0, 0.0).astype(np.float32)
    m8 = np.concatenate([ident.reshape(64, 256), trineg.reshape(64, 256)],
                        axis=1).astype(ml_dtypes.float8_e4m3fn)

    in_maps = []
    for cid in range(8):
        b, g = cid // 4, cid % 4
        # j-tile order: q(h0,h1) | k(h0,h1) | q(h2,h3) | k(h2,h3)
        qcols = [W_attn[:, g * V + h * D:g * V + (h + 1) * D] for h in range(HPC)]
        kcols = [W_attn[:, C + g * V + h * D:C + g * V + (h + 1) * D]
                 for h in range(HPC)]
        wqk = np.concatenate([qcols[0], qcols[1], kcols[0], kcols[1],
                              qcols[2], qcols[3], kcols[2], kcols[3]], axis=1)
        bq = [b_attn[g * V + h * D:g * V + (h + 1) * D] for h in range(HPC)]
        bk = [b_attn[C + g * V + h * D:C + g * V + (h + 1) * D]
              for h in range(HPC)]
        bqk = np.stack([np.concatenate([bq[0], bq[1]]),
                        np.concatenate([bk[0], bk[1]]),
                        np.concatenate([bq[2], bq[3]]),
                        np.concatenate([bk[2], bk[3]])], axis=1)  # [128, 4]
        in_maps.append({
            "xT": np.ascontiguousarray(x[b].T).astype(ml_dtypes.bfloat16),
            "w_qk": np.ascontiguousarray(wqk).astype(ml_dtypes.bfloat16),
            "b_qk": np.ascontiguousarray(bqk),
            "w_v": np.ascontiguousarray(
                W_attn[:, 2 * C + g * V:2 * C + (g + 1) * V])
                .astype(ml_dtypes.bfloat16),
            "b_v": np.ascontiguousarray(
                b_attn[2 * C + g * V:2 * C + (g + 1) * V].reshape(1, V)),
            "w_pr": np.ascontiguousarray(W_proj[g * V:(g + 1) * V, :])
                .astype(ml_dtypes.bfloat16),
            "m8": m8,
        })

    trace = os.environ.get("KTRACE") == "1"
    res = run_bass_kernel_spmd(nc, in_maps, core_ids=list(range(8)),
                               trace=trace)
    _cache["last_exec_ns"] = res.exec_time_ns
    _cache["last_result"] = res

    out = np.zeros((B, T, C), dtype=np.float32)
    for cid in range(8):
        out[cid // 4] += res.results[cid]["out"].astype(np.float32)
    out += b_proj[None, None, :]
    return out


# revision 18
# speedup vs baseline: 1.0603x; 1.0603x over previous
"""Causal self-attention (B=2, T=2048, C=1024, H=16) on 8 trn2 NeuronCores.

Sharding: core c handles batch b=c//4 and head group g=c%4 (4 heads each).
Data parallel on B, tensor parallel on H; W_attn/W_proj sliced per head
group; host sums the 4 tensor-parallel partial projection outputs per batch.

v2 schedule (vs the phase-serial v1): the kernel is bound by the PE's
streamed-column count (~274k cycles) with the ACT exp stream (~79us)
second; everything else must hide under those.  The p-state ramp makes any
PE stall expensive (half clock for ~3us), so emission order keeps the PE
queue saturated:
  - W_qk columns are reordered host-side so j-tiles come as
    [q(h0,h1) | k(h0,h1) | q(h2,h3) | k(h2,h3)]: attention for heads 0,1
    of pair 0 starts right after the first half of the qkT GEMM.
  - attention steps are software-pipelined S(g) -> exp(g) -> PV(g) with the
    remaining qkT j-tiles, the v GEMM groups, and pair-0 projection tiles
    injected between steps as PE filler while ACT chews exp.
  - one PSUM tag layout for all phases: 'big' [128,1024]x3 (qkT groups,
    v groups, S tiles, proj tiles) + 'y' [128,512]x2 (PV accumulators).
  - ACT runs ONLY Exp (one table load ever); all psum->sbuf copies and the
    qk bias add run on DVE (tensor_scalar_add with per-partition bias AP).
  - causal diag masking accumulates -448 into the mixed 128x128 block via an
    fp8e4 DoubleRow matmul (half the PE cycles of the bf16 version); exp's
    1/8 scale turns that into a -56 logit shift -> exact zeros.
  - S tail matmuls are exact-width (no widening), output is written bf16
    (halves the store DMA); host accumulates the 4 TP partials in f32.
Numerics match v1: bf16 operands, f32 psum, softmax denominator from a
ones column in v (psum row 0), reciprocal_approx_fast, no row-max.
"""

import os
import numpy as np
import ml_dtypes

import concourse.bacc as bacc
import concourse.mybir as mybir
import concourse.tile as tile
from concourse.bass_utils import run_bass_kernel_spmd
from concourse.masks import make_identity, make_lower_triangular

FP8_MASK = os.environ.get("KFP8MASK", "0") == "1"

B, T, C, H = 2, 2048, 1024, 16
D = C // H          # 64
HPC = H // 4        # 4 heads per core
QK = 2 * HPC * D    # 512 rows of qkT
V = HPC * D         # 256 v columns
F32 = mybir.dt.float32
BF16 = mybir.dt.bfloat16
FP8 = mybir.dt.float8e4
PAIR = 1024         # queries per attention pass
AF = mybir.ActivationFunctionType
DR = mybir.MatmulPerfMode.DoubleRow

_cache = {}


def _build():
    nc = bacc.Bacc("TRN2", target_bir_lowering=False, debug=False, num_devices=8)
    xT = nc.dram_tensor("xT", [C, T], BF16, kind="ExternalInput").ap()
    w_qk = nc.dram_tensor("w_qk", [C, QK], BF16, kind="ExternalInput").ap()
    b_qk = nc.dram_tensor("b_qk", [128, 4], F32, kind="ExternalInput").ap()
    w_v = nc.dram_tensor("w_v", [C, V], BF16, kind="ExternalInput").ap()
    b_v = nc.dram_tensor("b_v", [1, V], F32, kind="ExternalInput").ap()
    w_pr = nc.dram_tensor("w_pr", [V, C], BF16, kind="ExternalInput").ap()
    m8 = nc.dram_tensor("m8", [64, 512], FP8, kind="ExternalInput").ap()
    out = nc.dram_tensor("out", [T, C], BF16, kind="ExternalOutput").ap()

    NC_ = C // 128  # 8 c-tiles

    with tile.TileContext(nc) as tc:
        with (
            tc.tile_pool(name="const", bufs=1) as cpool,
            tc.tile_pool(name="xt", bufs=1) as xpool,
            tc.tile_pool(name="w", bufs=1) as wpool,
            tc.tile_pool(name="qk", bufs=1) as qkpool,
            tc.tile_pool(name="v", bufs=1) as vpool,
            tc.tile_pool(name="att", bufs=1) as apool,
            tc.tile_pool(name="o", bufs=1) as opool,
            tc.tile_pool(name="ps", bufs=1, space="PSUM") as ps,
        ):
            # ---- input DMA (each trigger costs ~600ns of sync-queue time,
            # so whole weights go as single strided transfers) ----
            wqk_sb = wpool.tile([128, NC_, QK], BF16, name="wqk")
            # j-tiles 0,1 first: the opening qkT groups only need them
            nc.sync.dma_start(wqk_sb[:, :, 0:256],
                              w_qk[:, 0:256].rearrange("(a p) j -> p a j", p=128))
            xt = []
            for c in range(NC_):
                t = xpool.tile([128, T], BF16, name=f"xt{c}")
                nc.sync.dma_start(t[:], xT[c * 128:(c + 1) * 128, :])
                xt.append(t)
                if c == 2:
                    bqk_sb = cpool.tile([128, 4], F32, name="bqk")
                    nc.sync.dma_start(bqk_sb[:], b_qk[:, :])
                    m8_sb = cpool.tile([64, 512], FP8, name="m8")
                    nc.sync.dma_start(m8_sb[:], m8[:, :])
                    bv_row = cpool.tile([1, V], F32, name="bv_row")
                    nc.sync.dma_start(bv_row[:], b_v[:])
            nc.sync.dma_start(wqk_sb[:, :, 256:512],
                              w_qk[:, 256:512].rearrange("(a p) j -> p a j", p=128))
            wv_sb = wpool.tile([128, NC_, V], BF16, name="wv")
            nc.sync.dma_start(wv_sb[:], w_v.rearrange("(a p) v -> p a v", p=128))
            wpr_sb = wpool.tile([128, 2, C], BF16, name="wpr")
            nc.sync.dma_start(wpr_sb[:], w_pr.rearrange("(k p) c -> p k c", p=128))

            if FP8_MASK:
                ident_pack = m8_sb[:, 0:256].rearrange("p (a c) -> p a c", a=2)
                trineg_pack = m8_sb[:, 256:512].rearrange("p (a c) -> p a c", a=2)
            else:
                ident_bf = cpool.tile([128, 128], BF16, name="ident_bf")
                make_identity(nc, ident_bf[:])
                trineg_bf = cpool.tile([128, 128], BF16, name="trineg_bf")
                make_lower_triangular(nc, trineg_bf[:], val=-8192.0, diag=False)
            bv_full = cpool.tile([128, V], F32, name="bv_full")
            nc.gpsimd.partition_broadcast(bv_full[:], bv_row[:])
            ones16 = cpool.tile([128, 16], F32, name="ones16")
            nc.gpsimd.memset(ones16[:], 1.0)

            qk_t = [qkpool.tile([128, T], BF16, name=f"qk{j}")
                    for j in range(QK // 128)]
            # v layout: 4 big tiles of 4 T-tiles each; per (tt,h) 128 cols:
            # col 0 = ones (softmax denominator -> psum row 0), cols 64..127
            # = v dims (y -> psum rows 64..127, 64-aligned).
            v_big = [vpool.tile([128, 4, HPC, 128], BF16, name=f"v{b_}")
                     for b_ in range(4)]
            for b_ in range(4):
                # cols 1..63 are never read downstream; zero them once so
                # the PV lhsT reads fully-initialized memory (CoreSim).
                nc.gpsimd.memset(v_big[b_][:, :, :, 1:64], 0.0)

            def qk_group(j, tp):
                pst = ps.tile([128, 1024], F32, name="qk_ps", tag="big", bufs=3)
                for c in range(NC_):
                    for s in range(2):
                        nc.tensor.matmul(
                            pst[:, s * 512:(s + 1) * 512],
                            wqk_sb[:, c, j * 128:(j + 1) * 128],
                            xt[c][:, tp * 1024 + s * 512:tp * 1024 + (s + 1) * 512],
                            start=(c == 0), stop=(c == NC_ - 1))
                nc.vector.tensor_scalar_add(
                    qk_t[j][:, tp * 1024:(tp + 1) * 1024], pst[:],
                    bqk_sb[:, j:j + 1])

            def v_group(vb):
                pst = ps.tile([128, 1024], F32, name="v_ps", tag="big", bufs=3)
                for tl in range(4):
                    tt = vb * 4 + tl
                    for c in range(NC_):
                        nc.tensor.matmul(
                            pst[:, tl * 256:(tl + 1) * 256],
                            xt[c][:, tt * 128:(tt + 1) * 128],
                            wv_sb[:, c, :],
                            start=(c == 0), stop=(c == NC_ - 1))
                for tl in range(4):
                    nc.vector.tensor_add(
                        v_big[vb][:, tl, :, 64:64 + D],
                        pst[:, tl * 256:(tl + 1) * 256]
                        .rearrange("p (h d) -> p h d", h=HPC),
                        bv_full[:].rearrange("p (h d) -> p h d", h=HPC))
                nc.vector.tensor_copy(
                    v_big[vb][:, :, :, 0:1],
                    ones16[:].rearrange("p (t h o) -> p t h o", t=4, h=HPC))

            def proj_group(p, tt):
                i0 = p * PAIR
                o_ps = ps.tile([128, 1024], F32, name="o_ps", tag="big", bufs=3)
                for cc in range(2):
                    for k in range(2):
                        nc.tensor.matmul(
                            o_ps[:, cc * 512:(cc + 1) * 512],
                            yn[p][k][:, tt * 128:(tt + 1) * 128],
                            wpr_sb[:, k, cc * 512:(cc + 1) * 512],
                            start=(k == 0), stop=(k == 1))
                osb = opool.tile([128, 1024], BF16, name="osb", tag="osb",
                                 bufs=3)
                nc.vector.tensor_copy(osb[:], o_ps[:])
                nc.sync.dma_start(
                    out[i0 + tt * 128:i0 + (tt + 1) * 128, :], osb[:])

            yn = {}

            def emit_s_exp(p, h, g):
                """S matmuls + exp for one (pair, head, j-tile); returns pT."""
                i0 = p * PAIR
                qrow = (h % 2) * D
                qtile = qk_t[2 * (h // 2)]
                ktile = qk_t[2 * (h // 2) + 1]
                j0 = g * 128
                dlt = max(0, j0 - i0)
                diag = j0 >= i0
                diag_sub = dlt // 512
                s_ps = ps.tile([128, PAIR], F32, name="s_ps", tag="big",
                               bufs=3)
                pT = apool.tile([128, PAIR], BF16, name="pT", tag="pT",
                                bufs=6)
                for sub in range(2):
                    lo = max(dlt, sub * 512)
                    if lo >= (sub + 1) * 512:
                        continue
                    nc.tensor.matmul(
                        s_ps[:, lo:(sub + 1) * 512],
                        ktile[qrow:qrow + D, j0:j0 + 128],
                        qtile[qrow:qrow + D, i0 + lo:i0 + (sub + 1) * 512],
                        start=True,
                        stop=not (diag and sub == diag_sub))
                if diag:
                    # accumulate a large negative value into the mixed
                    # diagonal block (ident.T @ trineg == trineg) so exp
                    # yields exact zeros above the diagonal.
                    if FP8_MASK:
                        nc.tensor.matmul(
                            s_ps[:, dlt:dlt + 128],
                            ident_pack, trineg_pack,
                            start=False, stop=True, perf_mode=DR)
                    else:
                        nc.tensor.matmul(
                            s_ps[:, dlt:dlt + 128],
                            ident_bf[:], trineg_bf[:],
                            start=False, stop=True)
                nc.scalar.activation(
                    pT[:, dlt:PAIR], s_ps[:, dlt:PAIR], AF.Exp,
                    scale=float(1.0 / np.sqrt(D)))
                return pT

            def emit_pv(p, h, g, pT, y_A, y_B):
                i0 = p * PAIR
                njt = (i0 + PAIR) // 128
                lastA = (i0 + 512) // 128 - 1
                dlt = max(0, g * 128 - i0)
                vsl = v_big[g // 4][:, g % 4, h, :]
                if dlt < 512:
                    nc.tensor.matmul(
                        y_A[:, dlt:512], vsl, pT[:, dlt:512],
                        start=(g == 0), stop=(g == lastA))
                loB = max(512, dlt)
                nc.tensor.matmul(
                    y_B[:, loB - 512:512], vsl, pT[:, loB:PAIR],
                    start=(g == 0), stop=(g == njt - 1))

            def alloc_y():
                return (ps.tile([128, 512], F32, name="y_A", tag="y", bufs=2),
                        ps.tile([128, 512], F32, name="y_B", tag="y", bufs=2))

            def emit_norm(p, h, y_A, y_B):
                # normalize: rows 64..127 divided by row 0 (l sums)
                qrow = (h % 2) * D
                rec = apool.tile([1, PAIR], F32, name="rec", tag="rec", bufs=2)
                nc.vector.reciprocal_approx_fast(rec[:, 0:512], y_A[0:1, :])
                nc.vector.reciprocal_approx_fast(rec[:, 512:PAIR], y_B[0:1, :])
                rb = apool.tile([D, PAIR], F32, name="rb", tag="rb", bufs=2)
                nc.gpsimd.partition_broadcast(rb[:, 0:512], rec[:, 0:512])
                nc.gpsimd.partition_broadcast(rb[:, 512:PAIR], rec[:, 512:PAIR])
                nc.vector.tensor_mul(
                    yn[p][h // 2][qrow:qrow + D, 0:512],
                    y_A[64:64 + D, :], rb[:, 0:512])
                nc.vector.tensor_mul(
                    yn[p][h // 2][qrow:qrow + D, 512:PAIR],
                    y_B[64:64 + D, :], rb[:, 512:PAIR])

            def attn_head_steps(p, h):
                """Yields once per j-tile step for (pair p, head h)."""
                njt = (p * PAIR + PAIR) // 128
                y_A, y_B = alloc_y()
                for g in range(njt):
                    pT = emit_s_exp(p, h, g)
                    emit_pv(p, h, g, pT, y_A, y_B)
                    yield
                emit_norm(p, h, y_A, y_B)

            def run_steps(heads, p, fillers, every):
                if 0 in heads:
                    yn[p] = [apool.tile([128, PAIR], BF16, name=f"yn{p}_{k}",
                                        tag=f"yn{k}", bufs=2)
                             for k in range(2)]
                n = 0
                for h in heads:
                    for _ in attn_head_steps(p, h):
                        n += 1
                        if fillers and n % every == 0:
                            fillers.pop(0)()
                while fillers:
                    fillers.pop(0)()

            # ================= emission =================
            # minimum front before attention(p0,h0) can run: q/k of heads
            # 0,1 for queries 0..1023 (tp=0 halves) + v tiles 0..7.  The
            # exp stream on ACT is the kernel's second-binding resource, so
            # it must start as early as possible and never starve; all
            # remaining GEMM work is injected between attention steps as
            # PE filler while ACT chews exp.
            qk_group(0, 0)
            qk_group(1, 0)
            v_group(0)
            v_group(1)
            fill0 = [lambda j=j: qk_group(j, 0) for j in (2, 3)]
            fill0 += [lambda vb=vb: v_group(vb) for vb in (2, 3)]
            run_steps([0, 1], 0, fill0, every=4)
            fill0b = [lambda j=j: qk_group(j, 1) for j in (0, 1, 2, 3)]
            run_steps([2, 3], 0, fill0b, every=4)
            # pair 1, all heads; fillers: pair-0 projection
            fill1 = [lambda tt=tt: proj_group(0, tt) for tt in range(8)]
            run_steps([0, 1, 2, 3], 1, fill1, every=8)
            for tt in range(8):
                proj_group(1, tt)
    nc.compile()
    return nc


def _get_nc():
    if "nc" not in _cache:
        _cache["nc"] = _build()
    return _cache["nc"]


def kernel(x, W_attn, b_attn, W_proj, b_proj):
    x = np.asarray(x, dtype=np.float32)
    W_attn = np.asarray(W_attn, dtype=np.float32)
    b_attn = np.asarray(b_attn, dtype=np.float32)
    W_proj = np.asarray(W_proj, dtype=np.float32)
    b_proj = np.asarray(b_proj, dtype=np.float32)

    nc = _get_nc()

    # fp8 packed causal-mask constants: ident_pack | trineg_pack, [64, 2,128]
    r = np.arange(64)[:, None, None]
    a = np.arange(2)[None, :, None]
    i = np.arange(128)[None, None, :]
    ident = (i == 64 * a + r).astype(np.float32)
    # -240 is the max-magnitude finite value in BOTH e4m3 variants; with
    # exp's 1/8 scale it is a -30 logit shift -> P ~ e-13, effectively zero.
    trineg = np.where(64 * a + r > i, -240.0, 0.0).astype(np.float32)
    m8 = np.concatenate([ident.reshape(64, 256), trineg.reshape(64, 256)],
                        axis=1).astype(ml_dtypes.float8_e4m3fn)

    in_maps = []
    for cid in range(8):
        b, g = cid // 4, cid % 4
        # j-tile order: q(h0,h1) | k(h0,h1) | q(h2,h3) | k(h2,h3)
        qcols = [W_attn[:, g * V + h * D:g * V + (h + 1) * D] for h in range(HPC)]
        kcols = [W_attn[:, C + g * V + h * D:C + g * V + (h + 1) * D]
                 for h in range(HPC)]
        wqk = np.concatenate([qcols[0], qcols[1], kcols[0], kcols[1],
                              qcols[2], qcols[3], kcols[2], kcols[3]], axis=1)
        bq = [b_attn[g * V + h * D:g * V + (h + 1) * D] for h in range(HPC)]
        bk = [b_attn[C + g * V + h * D:C + g * V + (h + 1) * D]
              for h in range(HPC)]
        bqk = np.stack([np.concatenate([bq[0], bq[1]]),
                        np.concatenate([bk[0], bk[1]]),
                        np.concatenate([bq[2], bq[3]]),
                        np.concatenate([bk[2], bk[3]])], axis=1)  # [128, 4]
        in_maps.append({
            "xT": np.ascontiguousarray(x[b].T).astype(ml_dtypes.bfloat16),
            "w_qk": np.ascontiguousarray(wqk).astype(ml_dtypes.bfloat16),
            "b_qk": np.ascontiguousarray(bqk),
            "w_v": np.ascontiguousarray(
                W_attn[:, 2 * C + g * V:2 * C + (g + 1) * V])
                .astype(ml_dtypes.bfloat16),
            "b_v": np.ascontiguousarray(
                b_attn[2 * C + g * V:2 * C + (g + 1) * V].reshape(1, V)),
            "w_pr": np.ascontiguousarray(W_proj[g * V:(g + 1) * V, :])
                .astype(ml_dtypes.bfloat16),
            "m8": m8,
        })

    trace = os.environ.get("KTRACE") == "1"
    res = run_bass_kernel_spmd(nc, in_maps, core_ids=list(range(8)),
                               trace=trace)
    _cache["last_exec_ns"] = res.exec_time_ns
    _cache["last_result"] = res

    out = np.zeros((B, T, C), dtype=np.float32)
    for cid in range(8):
        out[cid // 4] += res.results[cid]["out"].astype(np.float32)
    out += b_proj[None, None, :]
    return out
